# revision 22
# baseline (speedup 1.0000x reference)
"""Causal attention (B=8, N=4096, D=64) on 8 trn2 NeuronCores.

Sharding: batch b -> core b (data parallel, no cross-core comms).

Per-core kernel (flash-attention style, fully transposed dataflow):
  inputs (host pre-layouts):  qT [64, N], kT [64, N]   (d on partitions),
                              v_aug [128, N/128, 65]   (k-tiled; col 64 = 1.0;
                                                        padding-masked rows = 0)
  for each q-block (512 wide):
    for each causal k-tile PAIR (2 x 128 wide):
      logitsT[k, q]  = matmul(lhsT=kT_t  [64,128], rhs=qT_blk[64,512])  (PSUM)
      logitsT'[k, q] = matmul(lhsT=kT_t1 [64,128], rhs=qT_blk[64,512])  (PSUM)
      expT = exp(logitsT_pair * 1/sqrt(d))     one ACT op over [128,1024] ->SBUF
      if diagonal: expT half *= causal 0/1 mask tile                    (DVE)
      outT[d,q] (+)= matmul(lhsT=v_aug[128,65], rhs=expT_half[128,512]) (PSUM)
        -- v_aug col 64 is 1.0 => outT row 64 = softmax denominators
  per q-block: r = 1/outT[64]; bc = ones[64] (x) r (PE outer product);
               out = outT[0:64] * bc (DVE); DMA out -> outT_dram[:, q-block]
  host transposes outT_dram [64, N] back to [N, 64] at gather time.

Padding mask: host zeroes masked k rows of v_aug (incl. the ones column), so
masked keys contribute nothing to numerator or denominator -- exactly
equivalent to -inf logits.

All matmuls use float32r (full-rate fp32 on the PE at moving-dim >= 256).
"""

import os
from contextlib import ExitStack

import numpy as np

B, N, D = 8, 4096, 64
QBLK = 512
KTILE = 128

LAST_RESULTS = None
_NC_CACHE = {}


def build(n=N, d=D, qblk=QBLK, ktile=KTILE, lg_bufs=2, acc_bufs=3, pb_bufs=4):
    import concourse.bass as bass
    import concourse.mybir as mybir
    import concourse.tile as tile
    from concourse import bacc

    f32 = mybir.dt.float32
    f32r = mybir.dt.float32r
    nt = n // ktile          # number of k-tiles
    nqb = n // qblk          # number of q-blocks
    tpq = qblk // ktile      # k-tiles per q-block (diagonal span)
    assert tpq % 2 == 0

    nc = bacc.Bacc("TRN2", target_bir_lowering=False, debug=False,
                   enable_asserts=False)

    qk_d = nc.dram_tensor("qk", (d, nqb, 2, qblk), f32r,
                          kind="ExternalInput").ap()
    v_d = nc.dram_tensor("v_aug", (128, nt, d + 1), f32r,
                         kind="ExternalInput").ap()
    mk_d = nc.dram_tensor("cmasks", (128, tpq, qblk), f32r,
                          kind="ExternalInput").ap()
    on_d = nc.dram_tensor("ones_row", (1, d), f32r,
                          kind="ExternalInput").ap()
    oT_d = nc.dram_tensor("outT", (d, n), f32, kind="ExternalOutput").ap()

    scale = 1.0 / float(np.sqrt(d))

    with tile.TileContext(nc) as tc:
        with ExitStack() as ctx:
            singles = ctx.enter_context(tc.tile_pool(name="singles", bufs=1))
            pb_pool = ctx.enter_context(tc.tile_pool(name="pb", bufs=pb_bufs))
            small = ctx.enter_context(tc.tile_pool(name="small", bufs=2))
            ob_pool = ctx.enter_context(tc.tile_pool(name="ob", bufs=3))
            lg_pool = ctx.enter_context(
                tc.tile_pool(name="lg", bufs=lg_bufs, space="PSUM"))
            acc_pool = ctx.enter_context(
                tc.tile_pool(name="acc", bufs=acc_bufs, space="PSUM"))
            bc_pool = ctx.enter_context(
                tc.tile_pool(name="bc", bufs=1, space="PSUM"))

            # --- resident inputs -------------------------------------------
            qk_sb = singles.tile([d, nqb, 2, qblk], f32r)
            v_sb = singles.tile([128, nt, d + 1], f32r)
            ones_col = singles.tile([1, d], f32r)
            nc.sync.dma_start(out=ones_col, in_=on_d)
            mk_sb = singles.tile([128, tpq, qblk], f32r)
            nc.sync.dma_start(out=mk_sb, in_=mk_d)

            # chunked loads ordered by first use so compute starts early.
            # one packed (kT|qT) DMA per chunk => each matmul sees at most
            # one unobserved DMA semaphore (the self-loading f32r matmul
            # codegen only supports a single sync wait).
            vdma = []
            qkdma = []
            for c in range(nqb):
                qkdma.append(nc.sync.dma_start(
                    out=qk_sb[:, c, :, :], in_=qk_d[:, c, :, :]))
                vs, ve = c * tpq, min(nt, (c + 1) * tpq)
                vdma.append(nc.sync.dma_start(
                    out=v_sb[:, vs:ve, :], in_=v_d[:, vs:ve, :]))

            masks = [mk_sb[:, j, :] for j in range(tpq)]

            def kT_ap(t):
                c, r = divmod(t, tpq)
                return qk_sb[:, c, 0, r * ktile:(r + 1) * ktile]

            # --- main loop -------------------------------------------------
            def epilogue(acc, qs):
                # normalize: out = outT[0:64] / sums (sums = row d of acc)
                rsum = small.tile([1, qblk], f32r, name="rsum")
                with nc.allow_low_precision(
                        reason="f32r is bit-identical fp32 in SBUF"):
                    nc.vector.reciprocal(rsum, acc[d:d + 1, :])
                bc = bc_pool.tile([d, qblk], f32, name="bc")
                nc.tensor.matmul(
                    bc, lhsT=ones_col, rhs=rsum,
                    start=True, stop=True,
                )
                bc_sb = ob_pool.tile([d, qblk], f32, name="bc_sb")
                nc.vector.tensor_copy(bc_sb, bc)
                ob = ob_pool.tile([d, qblk], f32, name="ob")
                nc.vector.tensor_mul(ob, acc[0:d, :], bc_sb)
                nc.sync.dma_start(out=oT_d[:, qs:qs + qblk], in_=ob)

            pending = None  # software-pipelined epilogue (keeps PE fed)
            for qb in range(nqb):
                qs = qb * qblk
                q_sl = qk_sb[:, qb, 1, :]
                acc = acc_pool.tile([d + 1, qblk], f32, name="acc", tag="acc")
                npairs = (tpq * qb + tpq) // 2
                tlast = 2 * npairs - 1
                for p in range(npairs):
                    t0 = 2 * p
                    lg = lg_pool.tile([128, 2, qblk], f32, name="lg")
                    pb = pb_pool.tile([128, 2, qblk], f32r, name="pb")
                    for h in range(2):
                        t = t0 + h
                        nc.tensor.matmul(
                            lg[:, h, :],
                            lhsT=kT_ap(t),
                            rhs=q_sl,
                            start=True, stop=True,
                        )
                    nc.scalar.activation(
                        pb, lg, mybir.ActivationFunctionType.Exp,
                        scale=scale,
                    )
                    for h in range(2):
                        t = t0 + h
                        j = t - tpq * qb
                        if j >= 0:
                            nc.vector.tensor_mul(
                                pb[:, h, :], pb[:, h, :], masks[j])
                        nc.tensor.matmul(
                            acc,
                            lhsT=v_sb[:, t, :],
                            rhs=pb[:, h, :],
                            start=(t == 0), stop=(t == tlast),
                        )
                    if p == 0 and pending is not None:
                        epilogue(*pending)
                        pending = None
                pending = (acc, qs)
            epilogue(*pending)

    nc.compile()
    return nc


def _get_nc(key="main", **kw):
    if key not in _NC_CACHE:
        _NC_CACHE[key] = build(**kw)
    return _NC_CACHE[key]


def _prep_core_inputs(q, k, v, attn_mask, b, n=N, d=D, ktile=KTILE,
                      qblk=QBLK):
    nt = n // ktile
    nqb = n // qblk
    qT = q[b].T.astype(np.float32)          # [d, n]
    kT = k[b].T.astype(np.float32)
    qk = np.empty((d, nqb, 2, qblk), dtype=np.float32)
    qk[:, :, 0, :] = kT.reshape(d, nqb, qblk)
    qk[:, :, 1, :] = qT.reshape(d, nqb, qblk)
    v_aug = np.ones((n, d + 1), dtype=np.float32)
    v_aug[:, :d] = v[b]
    v_aug *= (attn_mask[b] != 0).astype(np.float32)[:, None]
    v_aug = np.ascontiguousarray(
        v_aug.reshape(nt, ktile, d + 1).transpose(1, 0, 2))
    tpq = qblk // ktile
    # causal 0/1 mask per diagonal alignment j: keep where q >= k + 128*j
    y = np.arange(qblk)[None, None, :]
    x = np.arange(ktile)[:, None, None]
    jj = np.arange(tpq)[None, :, None]
    cmasks = (y - x - ktile * jj >= 0).astype(np.float32)
    ones_row = np.ones((1, d), dtype=np.float32)
    return {"qk": qk, "v_aug": v_aug, "cmasks": cmasks, "ones_row": ones_row}


def kernel(q, k, v, attn_mask):
    global LAST_RESULTS
    q = np.asarray(q, dtype=np.float32)
    k = np.asarray(k, dtype=np.float32)
    v = np.asarray(v, dtype=np.float32)
    attn_mask = np.asarray(attn_mask)

    from concourse.bass_utils import run_bass_kernel_spmd

    nc = _get_nc()
    in_maps = [_prep_core_inputs(q, k, v, attn_mask, b) for b in range(B)]
    trace = bool(os.environ.get("BASS_TRACE"))
    LAST_RESULTS = run_bass_kernel_spmd(
        nc, in_maps, core_ids=list(range(B)), trace=trace)

    out = np.empty((B, N, D), dtype=np.float32)
    for b in range(B):
        out[b] = LAST_RESULTS.results[b]["outT"].T
    return out


# revision 24
# speedup vs baseline: 1.0752x; 1.0752x over previous
"""Causal attention (B=8, N=4096, D=64) on 8 trn2 NeuronCores.

Sharding: batch b -> core b (data parallel, no cross-core comms).

Per-core kernel (flash-attention style, fully transposed dataflow):
  inputs (host pre-layouts):  qT [64, N], kT [64, N]   (d on partitions),
                              v_aug [128, N/128, 65]   (k-tiled; col 64 = 1.0;
                                                        padding-masked rows = 0)
  for each q-block (512 wide):
    for each causal k-tile PAIR (2 x 128 wide):
      logitsT[k, q]  = matmul(lhsT=kT_t  [64,128], rhs=qT_blk[64,512])  (PSUM)
      logitsT'[k, q] = matmul(lhsT=kT_t1 [64,128], rhs=qT_blk[64,512])  (PSUM)
      expT = exp(logitsT_pair * 1/sqrt(d))     one ACT op over [128,1024] ->SBUF
      if diagonal: expT half *= causal 0/1 mask tile                    (DVE)
      outT[d,q] (+)= matmul(lhsT=v_aug[128,65], rhs=expT_half[128,512]) (PSUM)
        -- v_aug col 64 is 1.0 => outT row 64 = softmax denominators
  per q-block: r = 1/outT[64]; bc = ones[64] (x) r (PE outer product);
               out = outT[0:64] * bc (DVE); DMA out -> outT_dram[:, q-block]
  host transposes outT_dram [64, N] back to [N, 64] at gather time.

Padding mask: host zeroes masked k rows of v_aug (incl. the ones column), so
masked keys contribute nothing to numerator or denominator -- exactly
equivalent to -inf logits.

All matmuls use float32r (full-rate fp32 on the PE at moving-dim >= 256).
"""

import os
from contextlib import ExitStack

import numpy as np

B, N, D = 8, 4096, 64
QBLK = 512
KTILE = 128

LAST_RESULTS = None
_NC_CACHE = {}


def build(n=N, d=D, qblk=QBLK, ktile=KTILE, lg_bufs=2, acc_bufs=3, pb_bufs=4,
          op_dt="float16", epi_depth=2):
    import concourse.bass as bass
    import concourse.mybir as mybir
    import concourse.tile as tile
    from concourse import bacc

    f32 = mybir.dt.float32
    f32r = mybir.dt.float32r
    opd = getattr(mybir.dt, op_dt)   # matmul operand dtype (fp16 or f32r)
    nt = n // ktile          # number of k-tiles
    nqb = n // qblk          # number of q-blocks
    tpq = qblk // ktile      # k-tiles per q-block (diagonal span)
    assert tpq % 2 == 0

    nc = bacc.Bacc("TRN2", target_bir_lowering=False, debug=False,
                   enable_asserts=False)

    qk_d = nc.dram_tensor("qk", (d, nqb, 2, qblk), opd,
                          kind="ExternalInput").ap()
    v_d = nc.dram_tensor("v_aug", (128, nt, d + 1), opd,
                         kind="ExternalInput").ap()
    mk_d = nc.dram_tensor("cmasks", (128, tpq, qblk), opd,
                          kind="ExternalInput").ap()
    on_d = nc.dram_tensor("ones_row", (1, d), f32r,
                          kind="ExternalInput").ap()
    oT_d = nc.dram_tensor("outT", (d, n), f32, kind="ExternalOutput").ap()

    scale = 1.0 / float(np.sqrt(d))

    with tile.TileContext(nc) as tc:
        with ExitStack() as ctx:
            singles = ctx.enter_context(tc.tile_pool(name="singles", bufs=1))
            pb_pool = ctx.enter_context(tc.tile_pool(name="pb", bufs=pb_bufs))
            small = ctx.enter_context(tc.tile_pool(name="small", bufs=2))
            ob_pool = ctx.enter_context(tc.tile_pool(name="ob", bufs=3))
            lg_pool = ctx.enter_context(
                tc.tile_pool(name="lg", bufs=lg_bufs, space="PSUM"))
            acc_pool = ctx.enter_context(
                tc.tile_pool(name="acc", bufs=acc_bufs, space="PSUM"))
            bc_pool = ctx.enter_context(
                tc.tile_pool(name="bc", bufs=1, space="PSUM"))

            # --- resident inputs -------------------------------------------
            qk_sb = singles.tile([d, nqb, 2, qblk], opd)
            v_sb = singles.tile([128, nt, d + 1], opd)
            ones_col = singles.tile([1, d], f32r)
            nc.sync.dma_start(out=ones_col, in_=on_d)
            mk_sb = singles.tile([128, tpq, qblk], opd)
            nc.sync.dma_start(out=mk_sb, in_=mk_d)

            # chunked loads ordered by first use so compute starts early.
            # one packed (kT|qT) DMA per chunk => each matmul sees at most
            # one unobserved DMA semaphore (the self-loading f32r matmul
            # codegen only supports a single sync wait).
            vdma = []
            qkdma = []
            for c in range(nqb):
                qkdma.append(nc.sync.dma_start(
                    out=qk_sb[:, c, :, :], in_=qk_d[:, c, :, :]))
                vs, ve = c * tpq, min(nt, (c + 1) * tpq)
                vdma.append(nc.sync.dma_start(
                    out=v_sb[:, vs:ve, :], in_=v_d[:, vs:ve, :]))

            masks = [mk_sb[:, j, :] for j in range(tpq)]

            def kT_ap(t):
                c, r = divmod(t, tpq)
                return qk_sb[:, c, 0, r * ktile:(r + 1) * ktile]

            # --- main loop -------------------------------------------------
            def epilogue(acc, qs):
                # normalize: out = outT[0:64] / sums (sums = row d of acc)
                rsum = small.tile([1, qblk], f32r, name="rsum")
                with nc.allow_low_precision(
                        reason="f32r is bit-identical fp32 in SBUF"):
                    nc.vector.reciprocal(rsum, acc[d:d + 1, :])
                bc = bc_pool.tile([d, qblk], f32, name="bc")
                nc.tensor.matmul(
                    bc, lhsT=ones_col, rhs=rsum,
                    start=True, stop=True,
                )
                bc_sb = ob_pool.tile([d, qblk], f32, name="bc_sb")
                nc.vector.tensor_copy(bc_sb, bc)
                ob = ob_pool.tile([d, qblk], f32, name="ob")
                nc.vector.tensor_mul(ob, acc[0:d, :], bc_sb)
                nc.sync.dma_start(out=oT_d[:, qs:qs + qblk], in_=ob)

            pending = []  # software-pipelined epilogues (keep PE fed)
            for qb in range(nqb):
                qs = qb * qblk
                q_sl = qk_sb[:, qb, 1, :]
                acc = acc_pool.tile([d + 1, qblk], f32, name="acc", tag="acc")
                npairs = (tpq * qb + tpq) // 2
                tlast = 2 * npairs - 1
                for p in range(npairs):
                    t0 = 2 * p
                    lg = lg_pool.tile([128, 2, qblk], f32, name="lg")
                    pb = pb_pool.tile([128, 2, qblk], opd, name="pb")
                    for h in range(2):
                        t = t0 + h
                        nc.tensor.matmul(
                            lg[:, h, :],
                            lhsT=kT_ap(t),
                            rhs=q_sl,
                            start=True, stop=True,
                        )
                    nc.scalar.activation(
                        pb, lg, mybir.ActivationFunctionType.Exp,
                        scale=scale,
                    )
                    for h in range(2):
                        t = t0 + h
                        j = t - tpq * qb
                        if j >= 0:
                            nc.vector.tensor_mul(
                                pb[:, h, :], pb[:, h, :], masks[j])
                        nc.tensor.matmul(
                            acc,
                            lhsT=v_sb[:, t, :],
                            rhs=pb[:, h, :],
                            start=(t == 0), stop=(t == tlast),
                        )
                    if p == 0 and len(pending) >= epi_depth:
                        epilogue(*pending.pop(0))
                pending.append((acc, qs))
            for args in pending:
                epilogue(*args)

    nc.compile()
    return nc


def _get_nc(key="main", **kw):
    if key not in _NC_CACHE:
        _NC_CACHE[key] = build(**kw)
    return _NC_CACHE[key]


def _prep_core_inputs(q, k, v, attn_mask, b, n=N, d=D, ktile=KTILE,
                      qblk=QBLK, op_dt="float16"):
    npdt = np.float16 if op_dt == "float16" else np.float32
    nt = n // ktile
    nqb = n // qblk
    qT = q[b].T.astype(npdt)          # [d, n]
    kT = k[b].T.astype(npdt)
    qk = np.empty((d, nqb, 2, qblk), dtype=npdt)
    qk[:, :, 0, :] = kT.reshape(d, nqb, qblk)
    qk[:, :, 1, :] = qT.reshape(d, nqb, qblk)
    v_aug = np.ones((n, d + 1), dtype=np.float32)
    v_aug[:, :d] = v[b]
    v_aug *= (attn_mask[b] != 0).astype(np.float32)[:, None]
    v_aug = np.ascontiguousarray(
        v_aug.reshape(nt, ktile, d + 1).transpose(1, 0, 2)).astype(npdt)
    tpq = qblk // ktile
    # causal 0/1 mask per diagonal alignment j: keep where q >= k + 128*j
    y = np.arange(qblk)[None, None, :]
    x = np.arange(ktile)[:, None, None]
    jj = np.arange(tpq)[None, :, None]
    cmasks = (y - x - ktile * jj >= 0).astype(npdt)
    ones_row = np.ones((1, d), dtype=np.float32)
    return {"qk": qk, "v_aug": v_aug, "cmasks": cmasks, "ones_row": ones_row}


def kernel(q, k, v, attn_mask):
    global LAST_RESULTS
    q = np.asarray(q, dtype=np.float32)
    k = np.asarray(k, dtype=np.float32)
    v = np.asarray(v, dtype=np.float32)
    attn_mask = np.asarray(attn_mask)

    from concourse.bass_utils import run_bass_kernel_spmd

    nc = _get_nc()
    in_maps = [_prep_core_inputs(q, k, v, attn_mask, b) for b in range(B)]
    trace = bool(os.environ.get("BASS_TRACE"))
    LAST_RESULTS = run_bass_kernel_spmd(
        nc, in_maps, core_ids=list(range(B)), trace=trace)

    out = np.empty((B, N, D), dtype=np.float32)
    for b in range(B):
        out[b] = LAST_RESULTS.results[b]["outT"].T
    return out


# revision 25
# speedup vs baseline: 1.3037x; 1.2126x over previous
"""Causal attention (B=8, N=4096, D=64) on 8 trn2 NeuronCores.

Sharding: batch b -> core b (data parallel, no cross-core comms).

Per-core kernel (flash-attention style, fully transposed dataflow):
  inputs (host pre-layouts):  qT [64, N], kT [64, N]   (d on partitions),
                              v_aug [128, N/128, 65]   (k-tiled; col 64 = 1.0;
                                                        padding-masked rows = 0)
  for each q-block (512 wide):
    for each causal k-tile PAIR (2 x 128 wide):
      logitsT[k, q]  = matmul(lhsT=kT_t  [64,128], rhs=qT_blk[64,512])  (PSUM)
      logitsT'[k, q] = matmul(lhsT=kT_t1 [64,128], rhs=qT_blk[64,512])  (PSUM)
      expT = exp(logitsT_pair * 1/sqrt(d))     one ACT op over [128,1024] ->SBUF
      if diagonal: expT half *= causal 0/1 mask tile                    (DVE)
      outT[d,q] (+)= matmul(lhsT=v_aug[128,65], rhs=expT_half[128,512]) (PSUM)
        -- v_aug col 64 is 1.0 => outT row 64 = softmax denominators
  per q-block: r = 1/outT[64]; bc = ones[64] (x) r (PE outer product);
               out = outT[0:64] * bc (DVE); DMA out -> outT_dram[:, q-block]
  host transposes outT_dram [64, N] back to [N, 64] at gather time.

Padding mask: host zeroes masked k rows of v_aug (incl. the ones column), so
masked keys contribute nothing to numerator or denominator -- exactly
equivalent to -inf logits.

All matmuls use float32r (full-rate fp32 on the PE at moving-dim >= 256).
"""

import os
from contextlib import ExitStack

import numpy as np

B, N, D = 8, 4096, 64
QBLK = 512
KTILE = 128

LAST_RESULTS = None
_NC_CACHE = {}


def build(n=N, d=D, qblk=QBLK, ktile=KTILE, lg_bufs=3, acc_bufs=2, pb_bufs=6,
          op_dt="float16", epi_depth=1):
    import concourse.bass as bass
    import concourse.mybir as mybir
    import concourse.tile as tile
    from concourse import bacc

    f32 = mybir.dt.float32
    f32r = mybir.dt.float32r
    opd = getattr(mybir.dt, op_dt)   # matmul operand dtype (fp16 or f32r)
    nt = n // ktile          # number of k-tiles
    nqb = n // qblk          # number of q-blocks
    tpq = qblk // ktile      # k-tiles per q-block (diagonal span)
    assert tpq % 2 == 0

    nc = bacc.Bacc("TRN2", target_bir_lowering=False, debug=False,
                   enable_asserts=False)

    qk_d = nc.dram_tensor("qk", (d, nqb, 2, qblk), opd,
                          kind="ExternalInput").ap()
    v_d = nc.dram_tensor("v_aug", (128, nt, d + 1), opd,
                         kind="ExternalInput").ap()
    mk_d = nc.dram_tensor("cmasks", (128, tpq, qblk), opd,
                          kind="ExternalInput").ap()
    oT_d = nc.dram_tensor("outT", (d, n), f32, kind="ExternalOutput").ap()
    rs_d = nc.dram_tensor("rs_scratch", (nqb, qblk), f32,
                          kind="Internal").ap()

    scale = 1.0 / float(np.sqrt(d))

    with tile.TileContext(nc) as tc:
        with ExitStack() as ctx:
            singles = ctx.enter_context(tc.tile_pool(name="singles", bufs=1))
            pb_pool = ctx.enter_context(tc.tile_pool(name="pb", bufs=pb_bufs))
            small = ctx.enter_context(tc.tile_pool(name="small", bufs=2))
            ob_pool = ctx.enter_context(tc.tile_pool(name="ob", bufs=3))
            lg_pool = ctx.enter_context(
                tc.tile_pool(name="lg", bufs=lg_bufs, space="PSUM"))
            acc_pool = ctx.enter_context(
                tc.tile_pool(name="acc", bufs=acc_bufs, space="PSUM"))

            # --- resident inputs -------------------------------------------
            qk_sb = singles.tile([d, nqb, 2, qblk], opd)
            v_sb = singles.tile([128, nt, d + 1], opd)
            mk_sb = singles.tile([128, tpq, qblk], opd)
            nc.sync.dma_start(out=mk_sb, in_=mk_d)

            # chunked loads ordered by first use so compute starts early.
            # one packed (kT|qT) DMA per chunk => each matmul sees at most
            # one unobserved DMA semaphore (the self-loading f32r matmul
            # codegen only supports a single sync wait).
            vdma = []
            qkdma = []
            for c in range(nqb):
                qkdma.append(nc.sync.dma_start(
                    out=qk_sb[:, c, :, :], in_=qk_d[:, c, :, :]))
                vs, ve = c * tpq, min(nt, (c + 1) * tpq)
                vdma.append(nc.sync.dma_start(
                    out=v_sb[:, vs:ve, :], in_=v_d[:, vs:ve, :]))

            masks = [mk_sb[:, j, :] for j in range(tpq)]

            def kT_ap(t):
                c, r = divmod(t, tpq)
                return qk_sb[:, c, 0, r * ktile:(r + 1) * ktile]

            # --- main loop -------------------------------------------------
            def epilogue(acc, qs, qb):
                # normalize: out = outT[0:64] / sums (sums = row d of acc).
                # The per-q reciprocal is broadcast across partitions with a
                # DRAM round-trip (partition-step-0 reads are DRAM-only), so
                # the whole epilogue stays off the PE.
                rsum = small.tile([1, qblk], f32, name="rsum")
                nc.vector.reciprocal(rsum, acc[d:d + 1, :])
                nc.sync.dma_start(out=rs_d[qb:qb + 1, :], in_=rsum)
                rb = ob_pool.tile([d, qblk], f32, name="rb")
                rs_slice = rs_d[qb:qb + 1, :]
                brd = bass.AP(tensor=rs_slice.tensor, offset=rs_slice.offset,
                              ap=[[0, d], list(rs_slice.ap[-1])])
                nc.sync.dma_start(out=rb, in_=brd)
                ob = ob_pool.tile([d, qblk], f32, name="ob")
                nc.vector.tensor_mul(ob, acc[0:d, :], rb)
                nc.sync.dma_start(out=oT_d[:, qs:qs + qblk], in_=ob)

            pending = []  # software-pipelined epilogues (keep PE fed)
            for qb in range(nqb):
                qs = qb * qblk
                q_sl = qk_sb[:, qb, 1, :]
                acc = acc_pool.tile([d + 1, qblk], f32, name="acc", tag="acc")
                npairs = (tpq * qb + tpq) // 2
                tlast = 2 * npairs - 1
                for p in range(npairs):
                    t0 = 2 * p
                    lg = lg_pool.tile([128, 2, qblk], f32, name="lg")
                    pb = pb_pool.tile([128, 2, qblk], opd, name="pb")
                    for h in range(2):
                        t = t0 + h
                        nc.tensor.matmul(
                            lg[:, h, :],
                            lhsT=kT_ap(t),
                            rhs=q_sl,
                            start=True, stop=True,
                        )
                    nc.scalar.activation(
                        pb, lg, mybir.ActivationFunctionType.Exp,
                        scale=scale,
                    )
                    for h in range(2):
                        t = t0 + h
                        j = t - tpq * qb
                        if j >= 0:
                            nc.vector.tensor_mul(
                                pb[:, h, :], pb[:, h, :], masks[j])
                        nc.tensor.matmul(
                            acc,
                            lhsT=v_sb[:, t, :],
                            rhs=pb[:, h, :],
                            start=(t == 0), stop=(t == tlast),
                        )
                    if p == 0 and len(pending) >= epi_depth:
                        epilogue(*pending.pop(0))
                pending.append((acc, qs, qb))
            for args in pending:
                epilogue(*args)

    nc.compile()
    return nc


def _get_nc(key="main", **kw):
    if key not in _NC_CACHE:
        _NC_CACHE[key] = build(**kw)
    return _NC_CACHE[key]


def _prep_core_inputs(q, k, v, attn_mask, b, n=N, d=D, ktile=KTILE,
                      qblk=QBLK, op_dt="float16"):
    npdt = np.float16 if op_dt == "float16" else np.float32
    nt = n // ktile
    nqb = n // qblk
    qT = q[b].T.astype(npdt)          # [d, n]
    kT = k[b].T.astype(npdt)
    qk = np.empty((d, nqb, 2, qblk), dtype=npdt)
    qk[:, :, 0, :] = kT.reshape(d, nqb, qblk)
    qk[:, :, 1, :] = qT.reshape(d, nqb, qblk)
    v_aug = np.ones((n, d + 1), dtype=np.float32)
    v_aug[:, :d] = v[b]
    v_aug *= (attn_mask[b] != 0).astype(np.float32)[:, None]
    v_aug = np.ascontiguousarray(
        v_aug.reshape(nt, ktile, d + 1).transpose(1, 0, 2)).astype(npdt)
    tpq = qblk // ktile
    # causal 0/1 mask per diagonal alignment j: keep where q >= k + 128*j
    y = np.arange(qblk)[None, None, :]
    x = np.arange(ktile)[:, None, None]
    jj = np.arange(tpq)[None, :, None]
    cmasks = (y - x - ktile * jj >= 0).astype(npdt)
    return {"qk": qk, "v_aug": v_aug, "cmasks": cmasks}


def kernel(q, k, v, attn_mask):
    global LAST_RESULTS
    q = np.asarray(q, dtype=np.float32)
    k = np.asarray(k, dtype=np.float32)
    v = np.asarray(v, dtype=np.float32)
    attn_mask = np.asarray(attn_mask)

    from concourse.bass_utils import run_bass_kernel_spmd

    nc = _get_nc()
    in_maps = [_prep_core_inputs(q, k, v, attn_mask, b) for b in range(B)]
    trace = bool(os.environ.get("BASS_TRACE"))
    LAST_RESULTS = run_bass_kernel_spmd(
        nc, in_maps, core_ids=list(range(B)), trace=trace)

    out = np.empty((B, N, D), dtype=np.float32)
    for b in range(B):
        out[b] = LAST_RESULTS.results[b]["outT"].T
    return out


# revision 28
# speedup vs baseline: 1.6144x; 1.2383x over previous
"""Causal attention (B=8, N=4096, D=64) on 8 trn2 NeuronCores.

Sharding: batch b -> core b (data parallel, no cross-core comms).

Per-core kernel (flash-attention style, fully transposed dataflow):
  inputs (host pre-layouts):  qT [64, N], kT [64, N]   (d on partitions),
                              v_aug [128, N/128, 65]   (k-tiled; col 64 = 1.0;
                                                        padding-masked rows = 0)
  for each q-block (512 wide):
    for each causal k-tile PAIR (2 x 128 wide):
      logitsT[k, q]  = matmul(lhsT=kT_t  [64,128], rhs=qT_blk[64,512])  (PSUM)
      logitsT'[k, q] = matmul(lhsT=kT_t1 [64,128], rhs=qT_blk[64,512])  (PSUM)
      expT = exp(logitsT_pair * 1/sqrt(d))     one ACT op over [128,1024] ->SBUF
      if diagonal: expT half *= causal 0/1 mask tile                    (DVE)
      outT[d,q] (+)= matmul(lhsT=v_aug[128,65], rhs=expT_half[128,512]) (PSUM)
        -- v_aug col 64 is 1.0 => outT row 64 = softmax denominators
  per q-block: r = 1/outT[64]; bc = ones[64] (x) r (PE outer product);
               out = outT[0:64] * bc (DVE); DMA out -> outT_dram[:, q-block]
  host transposes outT_dram [64, N] back to [N, 64] at gather time.

Padding mask: host zeroes masked k rows of v_aug (incl. the ones column), so
masked keys contribute nothing to numerator or denominator -- exactly
equivalent to -inf logits.

All matmuls use float32r (full-rate fp32 on the PE at moving-dim >= 256).
"""

import os
from contextlib import ExitStack

import numpy as np

B, N, D = 8, 4096, 64
QBLK = 512
KTILE = 128

LAST_RESULTS = None
_NC_CACHE = {}


def build(n=N, d=D, qblk=QBLK, ktile=KTILE, lg_bufs=3, acc_bufs=2, pb_bufs=6,
          op_dt="float16", epi_depth=1):
    import concourse.bass as bass
    import concourse.mybir as mybir
    import concourse.tile as tile
    from concourse import bacc

    f32 = mybir.dt.float32
    f32r = mybir.dt.float32r
    opd = getattr(mybir.dt, op_dt)   # matmul operand dtype (fp16 or f32r)
    nt = n // ktile          # number of k-tiles
    nqb = n // qblk          # number of q-blocks
    tpq = qblk // ktile      # k-tiles per q-block (diagonal span)
    assert tpq % 2 == 0

    nc = bacc.Bacc("TRN2", target_bir_lowering=False, debug=False,
                   enable_asserts=False)

    qk_d = nc.dram_tensor("qk", (d, nqb, 2, qblk), opd,
                          kind="ExternalInput").ap()
    v_d = nc.dram_tensor("v_aug", (128, nt, d + 1), opd,
                         kind="ExternalInput").ap()
    mk_d = nc.dram_tensor("cmasks", (128, tpq, qblk), opd,
                          kind="ExternalInput").ap()
    oT_d = nc.dram_tensor("outT", (d, n), f32, kind="ExternalOutput").ap()
    rs_d = nc.dram_tensor("rs_scratch", (nqb, qblk), f32,
                          kind="Internal").ap()

    scale = 1.0 / float(np.sqrt(d))

    with tile.TileContext(nc) as tc:
        with ExitStack() as ctx:
            singles = ctx.enter_context(tc.tile_pool(name="singles", bufs=1))
            pb_pool = ctx.enter_context(tc.tile_pool(name="pb", bufs=pb_bufs))
            small = ctx.enter_context(tc.tile_pool(name="small", bufs=2))
            ob_pool = ctx.enter_context(tc.tile_pool(name="ob", bufs=3))
            lg_pool = ctx.enter_context(
                tc.tile_pool(name="lg", bufs=lg_bufs, space="PSUM"))
            acc_pool = ctx.enter_context(
                tc.tile_pool(name="acc", bufs=acc_bufs, space="PSUM"))

            # --- resident inputs -------------------------------------------
            qk_sb = singles.tile([d, nqb, 2, qblk], opd)
            v_sb = singles.tile([128, nt, d + 1], opd)
            mk_sb = singles.tile([128, tpq, qblk], opd)
            nc.sync.dma_start(out=mk_sb, in_=mk_d)

            # chunked loads ordered by first use so compute starts early.
            # one packed (kT|qT) DMA per chunk => each matmul sees at most
            # one unobserved DMA semaphore (the self-loading f32r matmul
            # codegen only supports a single sync wait).
            vdma = []
            qkdma = []
            for c in range(nqb):
                qkdma.append(nc.sync.dma_start(
                    out=qk_sb[:, c, :, :], in_=qk_d[:, c, :, :]))
                vs, ve = c * tpq, min(nt, (c + 1) * tpq)
                vdma.append(nc.sync.dma_start(
                    out=v_sb[:, vs:ve, :], in_=v_d[:, vs:ve, :]))

            masks = [mk_sb[:, j, :] for j in range(tpq)]

            def kT_ap(t):
                c, r = divmod(t, tpq)
                return qk_sb[:, c, 0, r * ktile:(r + 1) * ktile]

            # --- main loop -------------------------------------------------
            def epilogue(acc, qs, qb):
                # normalize: out = outT[0:64] / sums (sums = row d of acc).
                # The per-q reciprocal is broadcast across partitions with a
                # DRAM round-trip (partition-step-0 reads are DRAM-only), so
                # the whole epilogue stays off the PE.
                rsum = small.tile([1, qblk], f32, name="rsum")
                nc.vector.reciprocal(rsum, acc[d:d + 1, :])
                nc.sync.dma_start(out=rs_d[qb:qb + 1, :], in_=rsum)
                rb = ob_pool.tile([d, qblk], f32, name="rb")
                rs_slice = rs_d[qb:qb + 1, :]
                brd = bass.AP(tensor=rs_slice.tensor, offset=rs_slice.offset,
                              ap=[[0, d], list(rs_slice.ap[-1])])
                nc.sync.dma_start(out=rb, in_=brd)
                ob = ob_pool.tile([d, qblk], f32, name="ob")
                nc.vector.tensor_mul(ob, acc[0:d, :], rb)
                nc.sync.dma_start(out=oT_d[:, qs:qs + qblk], in_=ob)

            # Per global pair p: emit MM1s(p) + exp(p) [+ masks], then the
            # MM2s of pair p-1. This orders the PE stream as
            # [... MM1a(p) MM1b(p) MM2a(p-1) MM2b(p-1) ...] so the PE fills
            # the exp(p-1) latency with pair p's MM1s instead of stalling.
            pending = []   # software-pipelined epilogues
            mm2_q = []     # deferred MM2 emission: (acc, pb, t0, qb, tlast)

            def flush_mm2():
                acc_, pb_, t0_, qb_, tlast_ = mm2_q.pop(0)
                for h in range(2):
                    t = t0_ + h
                    nc.tensor.matmul(
                        acc_,
                        lhsT=v_sb[:, t, :],
                        rhs=pb_[:, h, :],
                        start=(t == 0), stop=(t == tlast_),
                    )
                if t0_ + 1 == tlast_:  # that was the last pair of q-block
                    if len(pending) >= epi_depth:
                        epilogue(*pending.pop(0))

            for qb in range(nqb):
                qs = qb * qblk
                q_sl = qk_sb[:, qb, 1, :]
                acc = acc_pool.tile([d + 1, qblk], f32, name="acc", tag="acc")
                npairs = (tpq * qb + tpq) // 2
                tlast = 2 * npairs - 1
                pending.append((acc, qs, qb))
                for p in range(npairs):
                    t0 = 2 * p
                    lg = lg_pool.tile([128, 2, qblk], f32, name="lg")
                    pb = pb_pool.tile([128, 2, qblk], opd, name="pb")
                    for h in range(2):
                        t = t0 + h
                        nc.tensor.matmul(
                            lg[:, h, :],
                            lhsT=kT_ap(t),
                            rhs=q_sl,
                            start=True, stop=True,
                        )
                    nc.scalar.activation(
                        pb, lg, mybir.ActivationFunctionType.Exp,
                        scale=scale,
                    )
                    for h in range(2):
                        j = t0 + h - tpq * qb
                        if j >= 0:
                            nc.vector.tensor_mul(
                                pb[:, h, :], pb[:, h, :], masks[j])
                    mm2_q.append((acc, pb, t0, qb, tlast))
                    if len(mm2_q) >= 2:
                        flush_mm2()
            while mm2_q:
                flush_mm2()
            for args in pending:
                epilogue(*args)

    nc.compile()
    return nc


def _get_nc(key="main", **kw):
    if key not in _NC_CACHE:
        _NC_CACHE[key] = build(**kw)
    return _NC_CACHE[key]


def _prep_core_inputs(q, k, v, attn_mask, b, n=N, d=D, ktile=KTILE,
                      qblk=QBLK, op_dt="float16"):
    npdt = np.float16 if op_dt == "float16" else np.float32
    nt = n // ktile
    nqb = n // qblk
    qT = q[b].T.astype(npdt)          # [d, n]
    kT = k[b].T.astype(npdt)
    qk = np.empty((d, nqb, 2, qblk), dtype=npdt)
    qk[:, :, 0, :] = kT.reshape(d, nqb, qblk)
    qk[:, :, 1, :] = qT.reshape(d, nqb, qblk)
    v_aug = np.ones((n, d + 1), dtype=np.float32)
    v_aug[:, :d] = v[b]
    v_aug *= (attn_mask[b] != 0).astype(np.float32)[:, None]
    v_aug = np.ascontiguousarray(
        v_aug.reshape(nt, ktile, d + 1).transpose(1, 0, 2)).astype(npdt)
    tpq = qblk // ktile
    # causal 0/1 mask per diagonal alignment j: keep where q >= k + 128*j
    y = np.arange(qblk)[None, None, :]
    x = np.arange(ktile)[:, None, None]
    jj = np.arange(tpq)[None, :, None]
    cmasks = (y - x - ktile * jj >= 0).astype(npdt)
    return {"qk": qk, "v_aug": v_aug, "cmasks": cmasks}


def kernel(q, k, v, attn_mask):
    global LAST_RESULTS
    q = np.asarray(q, dtype=np.float32)
    k = np.asarray(k, dtype=np.float32)
    v = np.asarray(v, dtype=np.float32)
    attn_mask = np.asarray(attn_mask)

    from concourse.bass_utils import run_bass_kernel_spmd

    nc = _get_nc()
    in_maps = [_prep_core_inputs(q, k, v, attn_mask, b) for b in range(B)]
    trace = bool(os.environ.get("BASS_TRACE"))
    LAST_RESULTS = run_bass_kernel_spmd(
        nc, in_maps, core_ids=list(range(B)), trace=trace)

    out = np.empty((B, N, D), dtype=np.float32)
    for b in range(B):
        out[b] = LAST_RESULTS.results[b]["outT"].T
    return out
